# revision 20
# baseline (speedup 1.0000x reference)
"""RetinaNet-style detection post-processing (decode + class-max + greedy NMS).

Device (8 NeuronCores, data-parallel over the anchor axis):
  - per-anchor max class logit (the memory-bound bulk: streams the [A,64]
    class predictions through a segmented DVE reduce)
  - box decode, with exp() built from vector float/int ops (Cephes-style,
    <=1 ulp vs the f32 reference) because the ACT table exp (~1e-5 rel) is
    not accurate enough for the NMS IoU threshold decisions.

Host (gather/unshard glue):
  - stable sort by (-max_logit, index) — reproduces the reference's
    argmax-with-tiebreak selection order exactly
  - greedy NMS over the sorted candidates (sequential by nature; the
    selection loop is tiny relative to the device-side streaming work)
  - sigmoid / per-class argmax only for the <=1000 kept rows.
"""

import sys
import numpy as np

for _p in ("/opt/trn_rl_repo", "/root/.axon_site/_ro/trn_rl_repo"):
    if _p not in sys.path:
        sys.path.append(_p)

A = 306900
C = 64
N_CORES = 8
R = 38400          # rows per core (core 7 overlaps core 6 by 300 rows)
STARTS = [0, 38400, 76800, 115200, 153600, 192000, 230400, 268500]
KR, TR = 50, 6     # class-reduce tiling: TR tiles of [128 partitions, KR rows, 64]
KD, TD = 150, 2    # decode tiling:       TD tiles of [128 partitions, KD rows, 4]

CLS_THRESH = 0.3
NMS_THRESH = np.float32(0.02)
MAX_DET = 1000

F32 = np.float32
# exp(x) = q(x)^4 with q ~= exp(x/4): degree-6 minimax on x in [-1.25, 1.25]
# (poly rel err 2.7e-9; after two squarings ~4.6e-7 vs the f32 reference —
# inside the empirically-verified safe band for the NMS IoU decisions).
# Leading coefficient first (x^6 .. x^0).
POLY = [3.3799582865867706e-07, 8.168392014340498e-06, 0.0001627641322556883,
        0.0026041450910270214, 0.03124999813735485, 0.25, 1.0]

_PROG = None      # (nc,) compiled bass program, built once per process
LAST_RESULTS = None
TRACE = False
TRACE_KW = {}


def _build_program():
    from contextlib import ExitStack
    import concourse.tile as tile
    from concourse import bacc, mybir

    f32 = mybir.dt.float32
    nc = bacc.Bacc("TRN2", target_bir_lowering=False, debug=False,
                   enable_asserts=False, num_devices=N_CORES)
    cls_d = nc.dram_tensor("cls_in", [R, C], f32, kind="ExternalInput").ap()
    loc_d = nc.dram_tensor("loc_in", [R, 4], f32, kind="ExternalInput").ap()
    anc_d = nc.dram_tensor("anc_in", [R, 4], f32, kind="ExternalInput").ap()
    m_d = nc.dram_tensor("m_out", [R], f32, kind="ExternalOutput").ap()
    box_d = nc.dram_tensor("boxes_out", [R, 4], f32, kind="ExternalOutput").ap()

    cls_v = cls_d.rearrange("(t p k) c -> t p k c", p=128, k=KR)
    m_v = m_d.rearrange("(t p k) -> t p k", p=128, k=KR)
    loc_v = loc_d.rearrange("(t p k) c -> t p k c", p=128, k=KD)
    anc_v = anc_d.rearrange("(t p k) c -> t p k c", p=128, k=KD)
    box_v = box_d.rearrange("(t p k) c -> t p k c", p=128, k=KD)

    AF = mybir.ActivationFunctionType
    X = mybir.AxisListType.X
    op = mybir.AluOpType

    with tile.TileContext(nc) as tc:
        with ExitStack() as ctx:
            cpool = ctx.enter_context(tc.tile_pool(name="cls", bufs=TR))
            mpool = ctx.enter_context(tc.tile_pool(name="m", bufs=TR))
            dpool = ctx.enter_context(tc.tile_pool(name="dec", bufs=2))
            wpool = ctx.enter_context(tc.tile_pool(name="wk", bufs=2))

            # ---- decode first: its (small) DMAs ride the GpSimd ring so the
            # Sync ring streams cls_in back-to-back; decode compute hides
            # under the cls stream.
            def w2(pool, tag):
                t_ = pool.tile([128, KD * 2], f32, tag=tag)
                return t_, t_[:].rearrange("p (k c) -> p k c", c=2)

            dec_loads = []
            for t in range(TD):
                lt = dpool.tile([128, KD * 4], f32, tag="loc")
                nc.sync.dma_start(lt[:].rearrange("p (k c) -> p k c", c=4), loc_v[t])
                at = dpool.tile([128, KD * 4], f32, tag="anc")
                nc.sync.dma_start(at[:].rearrange("p (k c) -> p k c", c=4), anc_v[t])
                dec_loads.append((lt, at))
            box_outs = []
            for t in range(TD):
                lt, at = dec_loads[t]
                l3 = lt[:].rearrange("p (k c) -> p k c", c=4)
                a3 = at[:].rearrange("p (k c) -> p k c", c=4)
                lw = l3[:, :, 2:4]           # exp args
                aw = a3[:, :, 2:4]
                axy = a3[:, :, 0:2]
                lxy = l3[:, :, 0:2]

                # exp(lw) = q(lw)^4, q = Horner deg-6. tensor*tensor on
                # GpSimd, affine (+const) steps on ACT (Copy = in*scale+bias
                # with immediates, bit-exact per-op rounding). Vector stays
                # pure-reduce: it's in-order, and decode ops queued there
                # would head-of-line block the reduce stream.
                pt, p3 = w2(wpool, "p1")
                nc.scalar.activation(p3, lw, AF.Copy, bias=POLY[1], scale=POLY[0])
                for ci in (2, 3, 4, 5, 6):
                    qt, q3 = w2(wpool, f"q{ci}")
                    nc.gpsimd.tensor_tensor(q3, p3, lw, op=op.mult)
                    pt, p3 = w2(wpool, f"p{ci}")
                    nc.scalar.activation(p3, q3, AF.Copy,
                                         bias=POLY[ci] if ci < 6 else 1.0, scale=1.0)
                sqt, sq3 = w2(wpool, "sq")
                nc.gpsimd.tensor_tensor(sq3, p3, p3, op=op.mult)
                et, e3 = w2(wpool, "e")
                nc.gpsimd.tensor_tensor(e3, sq3, sq3, op=op.mult)

                # decode
                xpt, xp3 = w2(wpool, "xp")
                nc.gpsimd.tensor_tensor(xp3, lxy, aw, op=op.mult)
                xyt, xy3 = w2(wpool, "xy")
                nc.gpsimd.tensor_tensor(xy3, xp3, axy, op=op.add)
                wht, wh3 = w2(wpool, "wh")
                nc.gpsimd.tensor_tensor(wh3, e3, aw, op=op.mult)
                hwt, hw3 = w2(wpool, "hw")
                nc.scalar.activation(hw3, wh3, AF.Copy, bias=0.0, scale=0.5)

                bt = dpool.tile([128, KD * 4], f32, tag="box")
                b3 = bt[:].rearrange("p (k c) -> p k c", c=4)
                nc.gpsimd.tensor_tensor(b3[:, :, 0:2], xy3, hw3, op=op.subtract)
                nc.gpsimd.tensor_tensor(b3[:, :, 2:4], xy3, hw3, op=op.add)
                box_outs.append((t, bt))

            # ---- class-max reduce: TR tiles, each [128, KR, 64] -> [128, KR]
            # cls pool holds all TR tiles so the Sync ring queues every load
            # upfront and streams at full HBM rate; outputs go last on the
            # same ring (Sync is idle by then, and this keeps DMA issues off
            # the busy compute engines).
            m_outs = []
            for t in range(TR):
                ct = cpool.tile([128, KR * C], f32, tag="cls")
                c3 = ct[:].rearrange("p (k c) -> p k c", c=C)
                mt = mpool.tile([128, KR], f32, tag="m")
                if t < TR - 1:
                    nc.sync.dma_start(c3, cls_v[t])
                    nc.vector.reduce_max(mt[:], c3, axis=X)
                else:
                    # split the last tile: half-size final reduce shortens the
                    # critical tail after the last DMA byte lands
                    h = KR // 2
                    nc.sync.dma_start(c3[:, :h], cls_v[t][:, :h])
                    nc.sync.dma_start(c3[:, h:], cls_v[t][:, h:])
                    nc.vector.reduce_max(mt[:, :h], c3[:, :h], axis=X)
                    nc.vector.reduce_max(mt[:, h:], c3[:, h:], axis=X)
                m_outs.append((t, mt))
            for t, mt in m_outs:
                nc.sync.dma_start(m_v[t], mt[:])
            for t, bt in box_outs:
                nc.sync.dma_start(box_v[t], bt[:].rearrange("p (k c) -> p k c", c=4))

    nc.compile()
    return nc


def _get_program():
    global _PROG
    if _PROG is None:
        _PROG = _build_program()
    return _PROG


def _run_device(loc, cls, anc):
    from concourse.bass_utils import run_bass_kernel_spmd
    nc = _get_program()
    in_maps = []
    for s in STARTS:
        in_maps.append({
            "cls_in": cls[s:s + R],
            "loc_in": loc[s:s + R],
            "anc_in": anc[s:s + R],
        })
    res = run_bass_kernel_spmd(nc, in_maps, list(range(N_CORES)),
                               trace=TRACE, **TRACE_KW)
    global LAST_RESULTS
    LAST_RESULTS = res
    m_full = np.empty(A, F32)
    boxes_full = np.empty((A, 4), F32)
    for i in range(7):
        s = STARTS[i]
        m_full[s:s + R] = res.results[i]["m_out"]
        boxes_full[s:s + R] = res.results[i]["boxes_out"]
    m_full[268800:] = res.results[7]["m_out"][300:]
    boxes_full[268800:] = res.results[7]["boxes_out"][300:]
    return m_full, boxes_full


def _greedy_nms(boxes, m_logit):
    """Greedy hard-NMS identical to the reference's argmax loop: stable sort
    by (-score, index), walk candidates, suppress by IoU > NMS_THRESH."""
    # score threshold (sigmoid is monotone: compare in f32 sigmoid space)
    s32 = (1.0 / (1.0 + np.exp(-m_logit.astype(np.float64)))).astype(F32)
    order = np.argsort(-m_logit, kind="stable").astype(np.int32)
    order = order[s32[order] > F32(CLS_THRESH)]
    n = len(order)
    if n == 0:
        return np.empty(0, np.int32)
    x1 = np.ascontiguousarray(boxes[order, 0]); y1 = np.ascontiguousarray(boxes[order, 1])
    x2 = np.ascontiguousarray(boxes[order, 2]); y2 = np.ascontiguousarray(boxes[order, 3])
    areas = (x2 - x1) * (y2 - y1)
    idx = order.copy()
    keep = np.empty(MAX_DET, np.int32)
    nk = 0
    while n > 0 and nk < MAX_DET:
        keep[nk] = idx[0]; nk += 1
        xx1 = np.maximum(x1[0], x1[:n]); yy1 = np.maximum(y1[0], y1[:n])
        xx2 = np.minimum(x2[0], x2[:n]); yy2 = np.minimum(y2[0], y2[:n])
        inter = np.clip(xx2 - xx1, 0, None) * np.clip(yy2 - yy1, 0, None)
        iou = inter / (areas[0] + areas[:n] - inter)
        mk = iou <= NMS_THRESH
        mk[0] = False
        sel = np.nonzero(mk)[0]
        n = len(sel)
        x1[:n] = x1[sel]; y1[:n] = y1[sel]; x2[:n] = x2[sel]; y2[:n] = y2[sel]
        areas[:n] = areas[sel]; idx[:n] = idx[sel]
    return keep[:nk]


def kernel(loc_pred, cls_pred, anchors):
    loc = np.ascontiguousarray(np.asarray(loc_pred, F32))
    cls = np.ascontiguousarray(np.asarray(cls_pred, F32))
    anc = np.ascontiguousarray(np.asarray(anchors, F32))
    assert cls.shape == (A, C) and loc.shape == (A, 4) and anc.shape == (A, 4)

    m_full, boxes_full = _run_device(loc, cls, anc)
    kept = _greedy_nms(boxes_full, m_full)
    nk = len(kept)

    keep_arr = np.full(MAX_DET, -1, np.int32)
    keep_arr[:nk] = kept
    valid = keep_arr >= 0
    idx = np.clip(keep_arr, 0, None)
    vf = valid.astype(F32)
    boxes_k = boxes_full[idx] * vf[:, None]
    scores = (1.0 / (1.0 + np.exp(-m_full[idx].astype(np.float64)))).astype(F32)
    scores_k = scores * vf
    labels_k = np.where(valid, cls[idx].argmax(axis=1).astype(np.int32), np.int32(-1))
    return boxes_k, labels_k.astype(np.int32), scores_k, valid


# revision 21
# speedup vs baseline: 1.0991x; 1.0991x over previous
"""RetinaNet-style detection post-processing (decode + class-max + greedy NMS).

Device (8 NeuronCores, data-parallel over the anchor axis):
  - per-anchor max class logit (the memory-bound bulk: streams the [A,64]
    class predictions through a segmented DVE reduce)
  - box decode, with exp(x) = q(x)^4 (q = degree-6 minimax for exp(x/4))
    built from elementwise ops (~5e-7 rel vs the f32 reference) because the
    ACT table exp (~1e-5 rel) is not accurate enough for the NMS IoU
    threshold decisions (empirically: 1e-6 box perturbations preserve the
    keep sequence, 1e-5 flips it).

Host (gather/unshard glue):
  - stable sort by (-max_logit, index) — reproduces the reference's
    argmax-with-tiebreak selection order exactly
  - greedy NMS over the sorted candidates (sequential by nature; the
    selection loop is tiny relative to the device-side streaming work)
  - sigmoid / per-class argmax only for the <=1000 kept rows.
"""

import sys
import numpy as np

for _p in ("/opt/trn_rl_repo", "/root/.axon_site/_ro/trn_rl_repo"):
    if _p not in sys.path:
        sys.path.append(_p)

A = 306900
C = 64
N_CORES = 8
R = 38400          # rows per core (core 7 overlaps core 6 by 300 rows)
STARTS = [0, 38400, 76800, 115200, 153600, 192000, 230400, 268500]
KR, TR = 50, 6     # class-reduce tiling: TR tiles of [128 partitions, KR rows, 64]
KD, TD = 150, 2    # decode tiling:       TD tiles of [128 partitions, KD rows, 4]

CLS_THRESH = 0.3
NMS_THRESH = np.float32(0.02)
MAX_DET = 1000

F32 = np.float32
# exp(x) = q(x)^4 with q ~= exp(x/4): degree-6 minimax on x in [-1.25, 1.25]
# (poly rel err 2.7e-9; after two squarings ~4.6e-7 vs the f32 reference —
# inside the empirically-verified safe band for the NMS IoU decisions).
# Leading coefficient first (x^6 .. x^0).
POLY = [3.3799582865867706e-07, 8.168392014340498e-06, 0.0001627641322556883,
        0.0026041450910270214, 0.03124999813735485, 0.25, 1.0]

_PROG = None      # (nc,) compiled bass program, built once per process
LAST_RESULTS = None
TRACE = False
TRACE_KW = {}


def _build_program():
    from contextlib import ExitStack
    import concourse.tile as tile
    from concourse import bacc, mybir

    f32 = mybir.dt.float32
    nc = bacc.Bacc("TRN2", target_bir_lowering=False, debug=False,
                   enable_asserts=False, num_devices=N_CORES)
    cls_d = nc.dram_tensor("cls_in", [R, C], f32, kind="ExternalInput").ap()
    loc_d = nc.dram_tensor("loc_in", [R, 4], f32, kind="ExternalInput").ap()
    anc_d = nc.dram_tensor("anc_in", [R, 4], f32, kind="ExternalInput").ap()
    m_d = nc.dram_tensor("m_out", [R], f32, kind="ExternalOutput").ap()
    box_d = nc.dram_tensor("boxes_out", [R, 4], f32, kind="ExternalOutput").ap()

    cls_v = cls_d.rearrange("(t p k) c -> t p k c", p=128, k=KR)
    m_v = m_d.rearrange("(t p k) -> t p k", p=128, k=KR)
    loc_v = loc_d.rearrange("(t p k) c -> t p k c", p=128, k=KD)
    anc_v = anc_d.rearrange("(t p k) c -> t p k c", p=128, k=KD)
    box_v = box_d.rearrange("(t p k) c -> t p k c", p=128, k=KD)

    AF = mybir.ActivationFunctionType
    X = mybir.AxisListType.X
    op = mybir.AluOpType

    with tile.TileContext(nc) as tc:
        with ExitStack() as ctx:
            cpool = ctx.enter_context(tc.tile_pool(name="cls", bufs=TR))
            mpool = ctx.enter_context(tc.tile_pool(name="m", bufs=TR))
            dpool = ctx.enter_context(tc.tile_pool(name="dec", bufs=2))
            wpool = ctx.enter_context(tc.tile_pool(name="wk", bufs=2))

            # ---- decode first: its (small) DMAs ride the GpSimd ring so the
            # Sync ring streams cls_in back-to-back; decode compute hides
            # under the cls stream.
            def w2(pool, tag):
                t_ = pool.tile([128, KD * 2], f32, tag=tag)
                return t_, t_[:].rearrange("p (k c) -> p k c", c=2)

            dec_loads = []
            for t in range(TD):
                lt = dpool.tile([128, KD * 4], f32, tag="loc")
                nc.sync.dma_start(lt[:].rearrange("p (k c) -> p k c", c=4), loc_v[t])
                at = dpool.tile([128, KD * 4], f32, tag="anc")
                nc.sync.dma_start(at[:].rearrange("p (k c) -> p k c", c=4), anc_v[t])
                dec_loads.append((lt, at))
            box_outs = []
            for t in range(TD):
                lt, at = dec_loads[t]
                l3 = lt[:].rearrange("p (k c) -> p k c", c=4)
                a3 = at[:].rearrange("p (k c) -> p k c", c=4)
                lw = l3[:, :, 2:4]           # exp args
                aw = a3[:, :, 2:4]
                axy = a3[:, :, 0:2]
                lxy = l3[:, :, 0:2]

                # exp(lw) = q(lw)^4, q = Horner deg-6. tensor*tensor on
                # GpSimd, affine (+const) steps on ACT (Copy = in*scale+bias
                # with immediates, bit-exact per-op rounding). Vector stays
                # pure-reduce: it's in-order, and decode ops queued there
                # would head-of-line block the reduce stream.
                pt, p3 = w2(wpool, "p1")
                nc.scalar.activation(p3, lw, AF.Copy, bias=POLY[1], scale=POLY[0])
                for ci in (2, 3, 4, 5, 6):
                    qt, q3 = w2(wpool, f"q{ci}")
                    nc.gpsimd.tensor_tensor(q3, p3, lw, op=op.mult)
                    pt, p3 = w2(wpool, f"p{ci}")
                    nc.scalar.activation(p3, q3, AF.Copy,
                                         bias=POLY[ci] if ci < 6 else 1.0, scale=1.0)
                sqt, sq3 = w2(wpool, "sq")
                nc.gpsimd.tensor_tensor(sq3, p3, p3, op=op.mult)
                et, e3 = w2(wpool, "e")
                nc.gpsimd.tensor_tensor(e3, sq3, sq3, op=op.mult)

                # decode
                xpt, xp3 = w2(wpool, "xp")
                nc.gpsimd.tensor_tensor(xp3, lxy, aw, op=op.mult)
                xyt, xy3 = w2(wpool, "xy")
                nc.gpsimd.tensor_tensor(xy3, xp3, axy, op=op.add)
                wht, wh3 = w2(wpool, "wh")
                nc.gpsimd.tensor_tensor(wh3, e3, aw, op=op.mult)
                hwt, hw3 = w2(wpool, "hw")
                nc.scalar.activation(hw3, wh3, AF.Copy, bias=0.0, scale=0.5)

                bt = dpool.tile([128, KD * 4], f32, tag="box")
                b3 = bt[:].rearrange("p (k c) -> p k c", c=4)
                nc.gpsimd.tensor_tensor(b3[:, :, 0:2], xy3, hw3, op=op.subtract)
                nc.gpsimd.tensor_tensor(b3[:, :, 2:4], xy3, hw3, op=op.add)
                box_outs.append((t, bt))

            # ---- class-max reduce: TR tiles, each [128, KR, 64] -> [128, KR]
            # cls pool holds all TR tiles so the Sync ring queues every load
            # upfront and streams at full HBM rate; outputs go last on the
            # same ring (Sync is idle by then, and this keeps DMA issues off
            # the busy compute engines).
            m_outs = []
            for t in range(TR):
                ct = cpool.tile([128, KR * C], f32, tag="cls")
                c3 = ct[:].rearrange("p (k c) -> p k c", c=C)
                mt = mpool.tile([128, KR], f32, tag="m")
                if t < TR - 1:
                    nc.sync.dma_start(c3, cls_v[t])
                    nc.vector.reduce_max(mt[:], c3, axis=X)
                else:
                    # split the last tile: half-size final reduce shortens the
                    # critical tail after the last DMA byte lands
                    h = KR // 2
                    nc.sync.dma_start(c3[:, :h], cls_v[t][:, :h])
                    nc.sync.dma_start(c3[:, h:], cls_v[t][:, h:])
                    nc.vector.reduce_max(mt[:, :h], c3[:, :h], axis=X)
                    nc.vector.reduce_max(mt[:, h:], c3[:, h:], axis=X)
                m_outs.append((t, mt))
            for t, mt in m_outs:
                nc.sync.dma_start(m_v[t], mt[:])
            for t, bt in box_outs:
                nc.sync.dma_start(box_v[t], bt[:].rearrange("p (k c) -> p k c", c=4))

    nc.compile()
    return nc


def _get_program():
    global _PROG
    if _PROG is None:
        _PROG = _build_program()
    return _PROG


def _run_device(loc, cls, anc):
    from concourse.bass_utils import run_bass_kernel_spmd
    nc = _get_program()
    in_maps = []
    for s in STARTS:
        in_maps.append({
            "cls_in": cls[s:s + R],
            "loc_in": loc[s:s + R],
            "anc_in": anc[s:s + R],
        })
    res = run_bass_kernel_spmd(nc, in_maps, list(range(N_CORES)),
                               trace=TRACE, **TRACE_KW)
    global LAST_RESULTS
    LAST_RESULTS = res
    m_full = np.empty(A, F32)
    boxes_full = np.empty((A, 4), F32)
    for i in range(7):
        s = STARTS[i]
        m_full[s:s + R] = res.results[i]["m_out"]
        boxes_full[s:s + R] = res.results[i]["boxes_out"]
    m_full[268800:] = res.results[7]["m_out"][300:]
    boxes_full[268800:] = res.results[7]["boxes_out"][300:]
    return m_full, boxes_full


def _greedy_nms(boxes, m_logit):
    """Greedy hard-NMS identical to the reference's argmax loop: stable sort
    by (-score, index), walk candidates, suppress by IoU > NMS_THRESH."""
    # score threshold (sigmoid is monotone: compare in f32 sigmoid space)
    s32 = (1.0 / (1.0 + np.exp(-m_logit.astype(np.float64)))).astype(F32)
    order = np.argsort(-m_logit, kind="stable").astype(np.int32)
    order = order[s32[order] > F32(CLS_THRESH)]
    n = len(order)
    if n == 0:
        return np.empty(0, np.int32)
    x1 = np.ascontiguousarray(boxes[order, 0]); y1 = np.ascontiguousarray(boxes[order, 1])
    x2 = np.ascontiguousarray(boxes[order, 2]); y2 = np.ascontiguousarray(boxes[order, 3])
    areas = (x2 - x1) * (y2 - y1)
    idx = order.copy()
    keep = np.empty(MAX_DET, np.int32)
    nk = 0
    while n > 0 and nk < MAX_DET:
        keep[nk] = idx[0]; nk += 1
        xx1 = np.maximum(x1[0], x1[:n]); yy1 = np.maximum(y1[0], y1[:n])
        xx2 = np.minimum(x2[0], x2[:n]); yy2 = np.minimum(y2[0], y2[:n])
        inter = np.clip(xx2 - xx1, 0, None) * np.clip(yy2 - yy1, 0, None)
        iou = inter / (areas[0] + areas[:n] - inter)
        mk = iou <= NMS_THRESH
        mk[0] = False
        sel = np.nonzero(mk)[0]
        n = len(sel)
        x1[:n] = x1[sel]; y1[:n] = y1[sel]; x2[:n] = x2[sel]; y2[:n] = y2[sel]
        areas[:n] = areas[sel]; idx[:n] = idx[sel]
    return keep[:nk]


def kernel(loc_pred, cls_pred, anchors):
    loc = np.ascontiguousarray(np.asarray(loc_pred, F32))
    cls = np.ascontiguousarray(np.asarray(cls_pred, F32))
    anc = np.ascontiguousarray(np.asarray(anchors, F32))
    assert cls.shape == (A, C) and loc.shape == (A, 4) and anc.shape == (A, 4)

    m_full, boxes_full = _run_device(loc, cls, anc)
    kept = _greedy_nms(boxes_full, m_full)
    nk = len(kept)

    keep_arr = np.full(MAX_DET, -1, np.int32)
    keep_arr[:nk] = kept
    valid = keep_arr >= 0
    idx = np.clip(keep_arr, 0, None)
    vf = valid.astype(F32)
    boxes_k = boxes_full[idx] * vf[:, None]
    scores = (1.0 / (1.0 + np.exp(-m_full[idx].astype(np.float64)))).astype(F32)
    scores_k = scores * vf
    labels_k = np.where(valid, cls[idx].argmax(axis=1).astype(np.int32), np.int32(-1))
    return boxes_k, labels_k.astype(np.int32), scores_k, valid
